# revision 16
# baseline (speedup 1.0000x reference)
"""Trainium2 Bass kernel for nn_CoordQuantizer (vq_codebook).

Strategy (8 NeuronCores, pure data-parallel over the 65536 tokens):
  - Host precomputes the (input-independent, key=42) gumbel factors
    hg = exp(g/2 - shift) once; TEMP=2 makes the gumbel softmax numerator
    exp((xc+g)/2) = exp(xc/2) * exp(g/2), so a single device-side exp pass
    u = exp(xc/2 - C) serves both paths:
        gumbel path : e_g  = u * hg
        KL path     : e_kl = u * u
  - Device layout is transposed ([codes, rows]) so the softmax axis sits on
    the partition dim and all reductions are matmul ones-columns:
        xcT  = vecsT_chunk.T @ xT          (8 chunks of 128 codes)
        qcT  = vecs_ext.T  @ e_g           (65 rows: 64 dims + sum(e_g))
        sklT = vecs_ext.T  @ e_kl          (65 rows: e_kl@vecs + sum(e_kl))
  - Host finishes scalar work: qc = qcT[:64]/qcT[64],
    KL rows from (x . skl)/s1 - log s1 - const, summed in f64.
"""

import os
from contextlib import ExitStack

import numpy as np

import concourse.bass as bass
import concourse.mybir as mybir
import concourse.tile as tile
from concourse import bacc
from concourse.bass_utils import run_bass_kernel_spmd

# ---- problem constants (hardcoded per contract) ----
B, S, D = 32, 2048, 64
R = B * S                      # 65536 rows
NCORES = 8
RC = R // NCORES               # 8192 rows per core
BLK = 512                      # rows per block
NBLK = RC // BLK               # 16
NCODE = 1000                   # coordinate codes
NPAD = 1024                    # padded to 8 chunks of 128
NCHUNK = NPAD // 128           # 8
K, N, E = 4, 8, 16             # codebooks / codes / dim
TEMP = 2.0
EPS = 1e-20
NUM_STEPS = 10
C3 = 10.0                      # u   = exp(xc/2 - C3)
C5 = 6.0                       # u_p = exp(xp/2 - C5)
LOG_U_C = float(np.log(1.0 / NUM_STEPS**3))
LOG_U_P = float(np.log(1.0 / N))

# ---- dtype knobs ----
F32 = mybir.dt.float32
_KNOBS = os.environ.get("KERNEL_DTYPES", "f32")
if _KNOBS == "16":
    # 16-bit elementwise path: u bf16, hg f16, matmul rhs + weights bf16
    HG_DT, U_DT, EG_DT, MMW_DT = (
        mybir.dt.float16,
        mybir.dt.bfloat16,
        mybir.dt.bfloat16,
        mybir.dt.bfloat16,
    )
    HG_NP = np.float16
elif _KNOBS == "mixed":
    # 16-bit DVE inputs, f32 DVE outputs + f32 matmuls
    HG_DT, U_DT, EG_DT, MMW_DT = mybir.dt.float16, mybir.dt.bfloat16, F32, F32
    HG_NP = np.float16
else:
    HG_DT, U_DT, EG_DT, MMW_DT = F32, F32, F32, F32
    HG_NP = np.float32

MMW_NP = mybir.dt.np(MMW_DT)

TRACE = False
LAST = {}

_cache = {}


def _gumbel_factors():
    """Per-core hg = exp(g/2 - shift) tensors, transposed+padded. Cached."""
    if "hg" in _cache:
        return _cache["hg"], _cache["hgp"], _cache["shift_c"], _cache["shift_p"]
    import jax
    import jax.numpy as jnp

    # Must reproduce the reference's gumbel draw bit-for-bit: use the ambient
    # jax defaults (this environment configures the rbg PRNG, whose bits are
    # backend-dependent) and the exact op sequence from reference._gumbel.
    kc, kp = jax.random.split(jax.random.key(42))
    uc = jax.random.uniform(kc, (B, S, NCODE), dtype=jnp.float32)
    gc = -jnp.log(-jnp.log(uc + EPS) + EPS)
    gc = np.asarray(gc).reshape(R, NCODE)
    up = jax.random.uniform(kp, (B, S, K, N), dtype=jnp.float32)
    gp = -jnp.log(-jnp.log(up + EPS) + EPS)
    gp = np.asarray(gp).reshape(R, K * N)

    # shift keeps exp(g/2 - shift) comfortably inside fp16 range; it cancels
    # in the softmax ratio so correctness is unaffected.
    shift_c = max(0.0, float(gc.max()) / 2.0 - 8.0)
    shift_p = max(0.0, float(gp.max()) / 2.0 - 8.0)

    hg_cores = []
    hgp_cores = []
    for c in range(NCORES):
        sl = slice(c * RC, (c + 1) * RC)
        hg = np.zeros((NPAD, RC), dtype=HG_NP)
        hg[:NCODE, :] = np.exp(gc[sl].T.astype(np.float64) / 2.0 - shift_c).astype(HG_NP)
        hg_cores.append(hg)
        hgp_cores.append(
            np.ascontiguousarray(
                np.exp(gp[sl].T.astype(np.float64) / 2.0 - shift_p)
            ).astype(HG_NP)
        )
    _cache["hg"] = hg_cores
    _cache["hgp"] = hgp_cores
    _cache["shift_c"] = shift_c
    _cache["shift_p"] = shift_p
    return hg_cores, hgp_cores, shift_c, shift_p


def _grid_flat():
    x = np.linspace(0.0, 1.5, NUM_STEPS)
    g = np.stack(np.meshgrid(x, x, x), axis=-1)
    return g.reshape(-1, 3).astype(np.float32)  # [1000, 3]


def _build_nc():
    if "nc" in _cache:
        return _cache["nc"]
    nc = bacc.Bacc()
    xT = nc.dram_tensor("xT", [D, RC], F32, kind="ExternalInput")
    hgT = nc.dram_tensor("hgT", [NPAD, RC], HG_DT, kind="ExternalInput")
    hgpT = nc.dram_tensor("hgpT", [K * N, RC], HG_DT, kind="ExternalInput")
    vT = nc.dram_tensor("vT", [D, NPAD], F32, kind="ExternalInput")
    vext = nc.dram_tensor("vext", [128, NCHUNK * 65], MMW_DT, kind="ExternalInput")
    wxp = nc.dram_tensor("wxp", [D, K * N], F32, kind="ExternalInput")
    wemb = nc.dram_tensor("wemb", [K * N, 68], MMW_DT, kind="ExternalInput")
    qcT_o = nc.dram_tensor("qcT_o", [65, RC], F32, kind="ExternalOutput")
    sklT_o = nc.dram_tensor("sklT_o", [65, RC], F32, kind="ExternalOutput")
    qpT_o = nc.dram_tensor("qpT_o", [68, RC], F32, kind="ExternalOutput")
    xpT_o = nc.dram_tensor("xpT_o", [K * N, RC], F32, kind="ExternalOutput")

    EXP = mybir.ActivationFunctionType.Exp

    with tile.TileContext(nc) as tc, ExitStack() as ctx:
        consts = ctx.enter_context(tc.tile_pool(name="consts", bufs=1))
        xpool = ctx.enter_context(tc.tile_pool(name="xpool", bufs=3))
        hgpool = ctx.enter_context(tc.tile_pool(name="hgpool", bufs=2))
        upool = ctx.enter_context(tc.tile_pool(name="upool", bufs=2))
        egpool = ctx.enter_context(tc.tile_pool(name="egpool", bufs=2))
        u2pool = ctx.enter_context(tc.tile_pool(name="u2pool", bufs=2))
        small = ctx.enter_context(tc.tile_pool(name="small", bufs=3))
        outp = ctx.enter_context(tc.tile_pool(name="outp", bufs=2))
        psxc = ctx.enter_context(tc.tile_pool(name="psxc", bufs=2, space="PSUM"))
        psqc = ctx.enter_context(tc.tile_pool(name="psqc", bufs=2, space="PSUM"))
        psskl = ctx.enter_context(tc.tile_pool(name="psskl", bufs=2, space="PSUM"))
        pspcb = ctx.enter_context(tc.tile_pool(name="pspcb", bufs=1, space="PSUM"))

        bias_c = consts.tile([128, 1], F32, tag="bias_c")
        nc.vector.memset(bias_c, -C3)
        bias_p = consts.tile([128, 1], F32, tag="bias_p")
        nc.vector.memset(bias_p, -C5)

        vT_t = consts.tile([D, NPAD], F32)
        nc.sync.dma_start(out=vT_t, in_=vT[:, :])
        vext_t = consts.tile([128, NCHUNK, 65], MMW_DT)
        nc.sync.dma_start(
            out=vext_t, in_=vext.rearrange("p (c w) -> p c w", c=NCHUNK)
        )
        wxp_t = consts.tile([D, K * N], F32)
        nc.sync.dma_start(out=wxp_t, in_=wxp[:, :])
        wemb_t = consts.tile([K * N, 68], MMW_DT)
        nc.sync.dma_start(out=wemb_t, in_=wemb[:, :])

        hgT_r = hgT.rearrange("(c p) r -> p c r", p=128)

        for b in range(NBLK):
            rs = b * BLK
            x_t = xpool.tile([D, BLK], F32, tag="x")
            nc.sync.dma_start(out=x_t, in_=xT[:, rs : rs + BLK])
            hg_t = hgpool.tile([128, NCHUNK, BLK], HG_DT, tag="hg")
            nc.sync.dma_start(out=hg_t, in_=hgT_r[:, :, rs : rs + BLK])

            u_t = upool.tile([128, NCHUNK, BLK], U_DT, tag="u")
            eg_t = egpool.tile([128, NCHUNK, BLK], EG_DT, tag="eg")
            u2_t = u2pool.tile([128, NCHUNK, BLK], EG_DT, tag="u2")
            qc_ps = psqc.tile([65, BLK], F32, tag="qc")
            skl_ps = psskl.tile([65, BLK], F32, tag="skl")

            for j in range(NCHUNK):
                xc_ps = psxc.tile([128, BLK], F32, tag="xc")
                nc.tensor.matmul(
                    xc_ps,
                    lhsT=vT_t[:, j * 128 : (j + 1) * 128],
                    rhs=x_t,
                    start=True,
                    stop=True,
                )
                nc.scalar.activation(
                    out=u_t[:, j, :], in_=xc_ps, func=EXP, bias=bias_c, scale=0.5
                )

            # full-block elementwise passes (one DVE op each)
            uf = u_t.rearrange("p c r -> p (c r)")
            nc.vector.tensor_mul(
                eg_t.rearrange("p c r -> p (c r)"),
                uf,
                hg_t.rearrange("p c r -> p (c r)"),
            )
            nc.vector.tensor_mul(u2_t.rearrange("p c r -> p (c r)"), uf, uf)

            for j in range(NCHUNK):
                nc.tensor.matmul(
                    qc_ps,
                    lhsT=vext_t[:, j, :],
                    rhs=eg_t[:, j, :],
                    start=(j == 0),
                    stop=(j == NCHUNK - 1),
                )
                nc.tensor.matmul(
                    skl_ps,
                    lhsT=vext_t[:, j, :],
                    rhs=u2_t[:, j, :],
                    start=(j == 0),
                    stop=(j == NCHUNK - 1),
                )

            qc_sb = outp.tile([65, BLK], F32, tag="qc_sb")
            nc.scalar.copy(out=qc_sb, in_=qc_ps)
            nc.sync.dma_start(out=qcT_o[:, rs : rs + BLK], in_=qc_sb)
            skl_sb = outp.tile([65, BLK], F32, tag="skl_sb")
            nc.vector.tensor_copy(out=skl_sb, in_=skl_ps)
            nc.sync.dma_start(out=sklT_o[:, rs : rs + BLK], in_=skl_sb)

            # ---- per-codebook path ----
            xp_ps = pspcb.tile([K * N, BLK], F32, tag="xp")
            nc.tensor.matmul(xp_ps, lhsT=wxp_t, rhs=x_t, start=True, stop=True)
            up_t = small.tile([K * N, BLK], U_DT, tag="up")
            nc.scalar.activation(
                out=up_t, in_=xp_ps, func=EXP, bias=bias_p[: K * N], scale=0.5
            )
            xp_sb = small.tile([K * N, BLK], F32, tag="xp_sb")
            nc.vector.tensor_copy(out=xp_sb, in_=xp_ps)
            nc.sync.dma_start(out=xpT_o[:, rs : rs + BLK], in_=xp_sb)
            hgp_t = small.tile([K * N, BLK], HG_DT, tag="hgp")
            nc.sync.dma_start(out=hgp_t, in_=hgpT[:, rs : rs + BLK])
            egp_t = small.tile([K * N, BLK], EG_DT, tag="egp")
            nc.vector.tensor_mul(egp_t, up_t, hgp_t)
            qp_ps = pspcb.tile([68, BLK], F32, tag="qp")
            nc.tensor.matmul(qp_ps, lhsT=wemb_t, rhs=egp_t, start=True, stop=True)
            qp_sb = outp.tile([68, BLK], F32, tag="qp_sb")
            nc.scalar.copy(out=qp_sb, in_=qp_ps)
            nc.sync.dma_start(out=qpT_o[:, rs : rs + BLK], in_=qp_sb)

    nc.finalize()
    _cache["nc"] = nc
    return nc


def kernel(inputs, linear_w, linear_b, emb_spaces, lin_ws):
    inputs = np.asarray(inputs, dtype=np.float32)
    linear_w = np.asarray(linear_w, dtype=np.float32)
    linear_b = np.asarray(linear_b, dtype=np.float32)
    emb_spaces = np.asarray(emb_spaces, dtype=np.float32)
    lin_ws = np.asarray(lin_ws, dtype=np.float32)

    hg_cores, hgp_cores, _, _ = _gumbel_factors()

    # vecs [1000, 64] = grid @ linear_w.T + linear_b
    vecs = _grid_flat() @ linear_w.T + linear_b[None, :]
    vT_np = np.zeros((D, NPAD), dtype=np.float32)
    vT_np[:, :NCODE] = vecs.T
    vext_full = np.zeros((NPAD, 65), dtype=np.float32)
    vext_full[:NCODE, :64] = vecs
    vext_full[:NCODE, 64] = 1.0
    vext_np = np.ascontiguousarray(
        vext_full.reshape(NCHUNK, 128, 65).transpose(1, 0, 2).reshape(128, NCHUNK * 65)
    ).astype(MMW_NP)

    embK = emb_spaces.reshape(K, N, E)
    w2 = np.einsum("kne,ked->knd", embK, lin_ws)  # [K,N,D]
    wxp_np = np.ascontiguousarray(w2.reshape(K * N, D).T)  # [64, 32]
    wemb_np = np.zeros((K * N, 68), dtype=np.float32)
    for k in range(K):
        wemb_np[k * N : (k + 1) * N, k * E : (k + 1) * E] = embK[k]
        wemb_np[k * N : (k + 1) * N, 64 + k] = 1.0
    wemb_np = wemb_np.astype(MMW_NP)

    x_flat = inputs.reshape(R, D)
    xT_cores = [
        np.ascontiguousarray(x_flat[c * RC : (c + 1) * RC].T) for c in range(NCORES)
    ]

    in_maps = [
        {
            "xT": xT_cores[c],
            "hgT": hg_cores[c],
            "hgpT": hgp_cores[c],
            "vT": vT_np,
            "vext": vext_np,
            "wxp": wxp_np,
            "wemb": wemb_np,
        }
        for c in range(NCORES)
    ]

    nc = _build_nc()
    res = run_bass_kernel_spmd(nc, in_maps, core_ids=list(range(NCORES)), trace=TRACE)
    LAST["exec_time_ns"] = res.exec_time_ns
    LAST["results"] = res

    quantized = np.empty((R, K * E), dtype=np.float32)
    quantized_coord = np.empty((R, D), dtype=np.float32)
    kl_c = 0.0
    kl_p = 0.0
    for c in range(NCORES):
        out = res.results[c]
        sl = slice(c * RC, (c + 1) * RC)

        qcT = out["qcT_o"].astype(np.float64)  # [65, RC]
        quantized_coord[sl] = (qcT[:64] / qcT[64:65]).T

        qpT = out["qpT_o"].astype(np.float64)  # [68, RC]
        se_p = qpT[64:68]  # [K, RC]
        qp = qpT[:64].reshape(K, E, RC) / se_p[:, None, :]
        quantized[sl] = qp.reshape(K * E, RC).T

        skl = out["sklT_o"].astype(np.float64)  # [65, RC]
        s1 = skl[64]
        s2 = np.einsum("dr,dr->r", xT_cores[c].astype(np.float64), skl[:64])
        # rows of kl_c: E_p[xc] - logsumexp - log_u
        kl_c += float(np.sum(s2 / s1 - (np.log(s1) + 2.0 * C3) - LOG_U_C))

        xp = out["xpT_o"].astype(np.float64).T.reshape(RC, K, N)
        m = xp.max(axis=-1, keepdims=True)
        ex = np.exp(xp - m)
        sp = ex.sum(axis=-1, keepdims=True)
        lse = m + np.log(sp)
        p = ex / sp
        kl_p += float(np.sum(p * (xp - lse - LOG_U_P)))

    loss = np.float32((kl_c + kl_p) / 5.0)
    return (
        quantized.reshape(B, S, K * E),
        quantized_coord.reshape(B, S, D),
        loss,
    )


# revision 19
# speedup vs baseline: 1.7192x; 1.7192x over previous
"""Trainium2 Bass kernel for nn_CoordQuantizer (vq_codebook).

Strategy (8 NeuronCores, pure data-parallel over the 65536 tokens):
  - Host precomputes the (input-independent, key=42) gumbel factors
    hg = exp(g/2 - shift) once; TEMP=2 makes the gumbel softmax numerator
    exp((xc+g)/2) = exp(xc/2) * exp(g/2), so a single device-side exp pass
    u = exp(xc/2 - C) serves both paths:
        gumbel path : e_g  = u * hg
        KL path     : e_kl = u * u
  - Device layout is transposed ([codes, rows]) so the softmax axis sits on
    the partition dim and all reductions are matmul ones-columns:
        xcT  = vecsT_chunk.T @ xT          (8 chunks of 128 codes)
        qcT  = vecs_ext.T  @ e_g           (65 rows: 64 dims + sum(e_g))
        sklT = vecs_ext.T  @ e_kl          (65 rows: e_kl@vecs + sum(e_kl))
  - Host finishes scalar work: qc = qcT[:64]/qcT[64],
    KL rows from (x . skl)/s1 - log s1 - const, summed in f64.
"""

import os
from contextlib import ExitStack

import numpy as np

import concourse.bass as bass
import concourse.mybir as mybir
import concourse.tile as tile
from concourse import bacc
from concourse.bass_utils import run_bass_kernel_spmd

# ---- problem constants (hardcoded per contract) ----
B, S, D = 32, 2048, 64
R = B * S                      # 65536 rows
NCORES = 8
RC = R // NCORES               # 8192 rows per core
BLK = 512                      # rows per block
NBLK = RC // BLK               # 16
NCODE = 1000                   # coordinate codes
NPAD = 1024                    # padded to 8 chunks of 128
NCHUNK = NPAD // 128           # 8
K, N, E = 4, 8, 16             # codebooks / codes / dim
TEMP = 2.0
EPS = 1e-20
NUM_STEPS = 10
C3 = 10.0                      # u   = exp(xc/2 - C3)
C5 = 6.0                       # u_p = exp(xp/2 - C5)
LOG_U_C = float(np.log(1.0 / NUM_STEPS**3))
LOG_U_P = float(np.log(1.0 / N))

# ---- dtype knobs ----
F32 = mybir.dt.float32
_KNOBS = os.environ.get("KERNEL_DTYPES", "f32")
if _KNOBS == "16":
    # 16-bit elementwise path: u bf16, hg f16, matmul rhs + weights bf16
    HG_DT, U_DT, EG_DT, MMW_DT = (
        mybir.dt.float16,
        mybir.dt.bfloat16,
        mybir.dt.bfloat16,
        mybir.dt.bfloat16,
    )
    HG_NP = np.float16
elif _KNOBS == "mixed":
    # 16-bit DVE inputs, f32 DVE outputs + f32 matmuls
    HG_DT, U_DT, EG_DT, MMW_DT = mybir.dt.float16, mybir.dt.bfloat16, F32, F32
    HG_NP = np.float16
else:
    HG_DT, U_DT, EG_DT, MMW_DT = F32, F32, F32, F32
    HG_NP = np.float32

MMW_NP = mybir.dt.np(MMW_DT)

# float32r runs the PE at full rate (vs 4 cyc/row for float32) for moving
# dims >= 256; everything feeding a matmul is declared float32r (same bytes
# on the host side, hardware rounds producer outputs).
if os.environ.get("KERNEL_F32R", "1") == "1":
    MM_IN = mybir.dt.float32r
    if MMW_DT == F32:
        MMW_DT = mybir.dt.float32r
    if EG_DT == F32:
        EG_DT = mybir.dt.float32r
else:
    MM_IN = F32

TRACE = False
LAST = {}

_cache = {}


def _gumbel_factors():
    """Per-core hg = exp(g/2 - shift) tensors, transposed+padded. Cached."""
    if "hg" in _cache:
        return _cache["hg"], _cache["hgp"], _cache["shift_c"], _cache["shift_p"]
    import jax
    import jax.numpy as jnp

    # Must reproduce the reference's gumbel draw bit-for-bit: use the ambient
    # jax defaults (this environment configures the rbg PRNG, whose bits are
    # backend-dependent) and the exact op sequence from reference._gumbel.
    kc, kp = jax.random.split(jax.random.key(42))
    uc = jax.random.uniform(kc, (B, S, NCODE), dtype=jnp.float32)
    gc = -jnp.log(-jnp.log(uc + EPS) + EPS)
    gc = np.asarray(gc).reshape(R, NCODE)
    up = jax.random.uniform(kp, (B, S, K, N), dtype=jnp.float32)
    gp = -jnp.log(-jnp.log(up + EPS) + EPS)
    gp = np.asarray(gp).reshape(R, K * N)

    # shift keeps exp(g/2 - shift) comfortably inside fp16 range; it cancels
    # in the softmax ratio so correctness is unaffected.
    shift_c = max(0.0, float(gc.max()) / 2.0 - 8.0)
    shift_p = max(0.0, float(gp.max()) / 2.0 - 8.0)

    hg_cores = []
    hgp_cores = []
    for c in range(NCORES):
        sl = slice(c * RC, (c + 1) * RC)
        hg = np.zeros((NPAD, RC), dtype=HG_NP)
        hg[:NCODE, :] = np.exp(gc[sl].T.astype(np.float64) / 2.0 - shift_c).astype(HG_NP)
        hg_cores.append(hg)
        hgp_cores.append(
            np.ascontiguousarray(
                np.exp(gp[sl].T.astype(np.float64) / 2.0 - shift_p)
            ).astype(HG_NP)
        )
    _cache["hg"] = hg_cores
    _cache["hgp"] = hgp_cores
    _cache["shift_c"] = shift_c
    _cache["shift_p"] = shift_p
    return hg_cores, hgp_cores, shift_c, shift_p


def _grid_flat():
    x = np.linspace(0.0, 1.5, NUM_STEPS)
    g = np.stack(np.meshgrid(x, x, x), axis=-1)
    return g.reshape(-1, 3).astype(np.float32)  # [1000, 3]


def _build_nc():
    if "nc" in _cache:
        return _cache["nc"]
    nc = bacc.Bacc()
    xT = nc.dram_tensor("xT", [D, RC], MM_IN, kind="ExternalInput")
    hgT = nc.dram_tensor("hgT", [NPAD, RC], HG_DT, kind="ExternalInput")
    hgpT = nc.dram_tensor("hgpT", [K * N, RC], HG_DT, kind="ExternalInput")
    vT = nc.dram_tensor("vT", [D, NPAD], MM_IN, kind="ExternalInput")
    vext = nc.dram_tensor("vext", [128, NCHUNK * 65], MMW_DT, kind="ExternalInput")
    wxp = nc.dram_tensor("wxp", [D, K * N], MM_IN, kind="ExternalInput")
    wemb = nc.dram_tensor("wemb", [K * N, 68], MMW_DT, kind="ExternalInput")
    qcT_o = nc.dram_tensor("qcT_o", [65, RC], F32, kind="ExternalOutput")
    sklT_o = nc.dram_tensor("sklT_o", [65, RC], F32, kind="ExternalOutput")
    qpT_o = nc.dram_tensor("qpT_o", [68, RC], F32, kind="ExternalOutput")
    xpT_o = nc.dram_tensor("xpT_o", [K * N, RC], F32, kind="ExternalOutput")

    EXP = mybir.ActivationFunctionType.Exp

    with tile.TileContext(nc) as tc, ExitStack() as ctx:
        consts = ctx.enter_context(tc.tile_pool(name="consts", bufs=1))
        xpool = ctx.enter_context(tc.tile_pool(name="xpool", bufs=3))
        hgpool = ctx.enter_context(tc.tile_pool(name="hgpool", bufs=2))
        upool = ctx.enter_context(tc.tile_pool(name="upool", bufs=2))
        egpool = ctx.enter_context(tc.tile_pool(name="egpool", bufs=2))
        u2pool = ctx.enter_context(tc.tile_pool(name="u2pool", bufs=2))
        small = ctx.enter_context(tc.tile_pool(name="small", bufs=3))
        outp = ctx.enter_context(tc.tile_pool(name="outp", bufs=2))
        psxc = ctx.enter_context(tc.tile_pool(name="psxc", bufs=2, space="PSUM"))
        psqc = ctx.enter_context(tc.tile_pool(name="psqc", bufs=2, space="PSUM"))
        psskl = ctx.enter_context(tc.tile_pool(name="psskl", bufs=2, space="PSUM"))
        pspcb = ctx.enter_context(tc.tile_pool(name="pspcb", bufs=1, space="PSUM"))

        bias_c = consts.tile([128, 1], F32, tag="bias_c")
        nc.vector.memset(bias_c, -C3)
        bias_p = consts.tile([128, 1], F32, tag="bias_p")
        nc.vector.memset(bias_p, -C5)

        vT_t = consts.tile([D, NPAD], MM_IN)
        nc.sync.dma_start(out=vT_t, in_=vT[:, :])
        vext_t = consts.tile([128, NCHUNK, 65], MMW_DT)
        nc.sync.dma_start(
            out=vext_t, in_=vext.rearrange("p (c w) -> p c w", c=NCHUNK)
        )
        wxp_t = consts.tile([D, K * N], MM_IN)
        nc.sync.dma_start(out=wxp_t, in_=wxp[:, :])
        wemb_t = consts.tile([K * N, 68], MMW_DT)
        nc.sync.dma_start(out=wemb_t, in_=wemb[:, :])

        hgT_r = hgT.rearrange("(c p) r -> p c r", p=128)

        for b in range(NBLK):
            rs = b * BLK
            x_t = xpool.tile([D, BLK], MM_IN, tag="x")
            nc.sync.dma_start(out=x_t, in_=xT[:, rs : rs + BLK])
            hg_t = hgpool.tile([128, NCHUNK, BLK], HG_DT, tag="hg")
            nc.sync.dma_start(out=hg_t, in_=hgT_r[:, :, rs : rs + BLK])

            u_t = upool.tile([128, NCHUNK, BLK], U_DT, tag="u")
            eg_t = egpool.tile([128, NCHUNK, BLK], EG_DT, tag="eg")
            u2_t = u2pool.tile([128, NCHUNK, BLK], EG_DT, tag="u2")
            qc_ps = psqc.tile([65, BLK], F32, tag="qc")
            skl_ps = psskl.tile([65, BLK], F32, tag="skl")

            for j in range(NCHUNK):
                xc_ps = psxc.tile([128, BLK], F32, tag="xc")
                nc.tensor.matmul(
                    xc_ps,
                    lhsT=vT_t[:, j * 128 : (j + 1) * 128],
                    rhs=x_t,
                    start=True,
                    stop=True,
                )
                nc.scalar.activation(
                    out=u_t[:, j, :], in_=xc_ps, func=EXP, bias=bias_c, scale=0.5
                )

            # full-block elementwise passes (one DVE op each)
            uf = u_t.rearrange("p c r -> p (c r)")
            nc.vector.tensor_mul(
                eg_t.rearrange("p c r -> p (c r)"),
                uf,
                hg_t.rearrange("p c r -> p (c r)"),
            )
            nc.vector.tensor_mul(u2_t.rearrange("p c r -> p (c r)"), uf, uf)

            for j in range(NCHUNK):
                nc.tensor.matmul(
                    qc_ps,
                    lhsT=vext_t[:, j, :],
                    rhs=eg_t[:, j, :],
                    start=(j == 0),
                    stop=(j == NCHUNK - 1),
                )
                nc.tensor.matmul(
                    skl_ps,
                    lhsT=vext_t[:, j, :],
                    rhs=u2_t[:, j, :],
                    start=(j == 0),
                    stop=(j == NCHUNK - 1),
                )

            qc_sb = outp.tile([65, BLK], F32, tag="qc_sb")
            nc.scalar.copy(out=qc_sb, in_=qc_ps)
            nc.sync.dma_start(out=qcT_o[:, rs : rs + BLK], in_=qc_sb)
            skl_sb = outp.tile([65, BLK], F32, tag="skl_sb")
            nc.vector.tensor_copy(out=skl_sb, in_=skl_ps)
            nc.sync.dma_start(out=sklT_o[:, rs : rs + BLK], in_=skl_sb)

            # ---- per-codebook path ----
            xp_ps = pspcb.tile([K * N, BLK], F32, tag="xp")
            nc.tensor.matmul(xp_ps, lhsT=wxp_t, rhs=x_t, start=True, stop=True)
            up_t = small.tile([K * N, BLK], U_DT, tag="up")
            nc.scalar.activation(
                out=up_t, in_=xp_ps, func=EXP, bias=bias_p[: K * N], scale=0.5
            )
            xp_sb = small.tile([K * N, BLK], F32, tag="xp_sb")
            nc.vector.tensor_copy(out=xp_sb, in_=xp_ps)
            nc.sync.dma_start(out=xpT_o[:, rs : rs + BLK], in_=xp_sb)
            hgp_t = small.tile([K * N, BLK], HG_DT, tag="hgp")
            nc.sync.dma_start(out=hgp_t, in_=hgpT[:, rs : rs + BLK])
            egp_t = small.tile([K * N, BLK], EG_DT, tag="egp")
            nc.vector.tensor_mul(egp_t, up_t, hgp_t)
            qp_ps = pspcb.tile([68, BLK], F32, tag="qp")
            nc.tensor.matmul(qp_ps, lhsT=wemb_t, rhs=egp_t, start=True, stop=True)
            qp_sb = outp.tile([68, BLK], F32, tag="qp_sb")
            nc.scalar.copy(out=qp_sb, in_=qp_ps)
            nc.sync.dma_start(out=qpT_o[:, rs : rs + BLK], in_=qp_sb)

    nc.finalize()
    _cache["nc"] = nc
    return nc


def kernel(inputs, linear_w, linear_b, emb_spaces, lin_ws):
    inputs = np.asarray(inputs, dtype=np.float32)
    linear_w = np.asarray(linear_w, dtype=np.float32)
    linear_b = np.asarray(linear_b, dtype=np.float32)
    emb_spaces = np.asarray(emb_spaces, dtype=np.float32)
    lin_ws = np.asarray(lin_ws, dtype=np.float32)

    hg_cores, hgp_cores, _, _ = _gumbel_factors()

    # vecs [1000, 64] = grid @ linear_w.T + linear_b
    vecs = _grid_flat() @ linear_w.T + linear_b[None, :]
    vT_np = np.zeros((D, NPAD), dtype=np.float32)
    vT_np[:, :NCODE] = vecs.T
    vext_full = np.zeros((NPAD, 65), dtype=np.float32)
    vext_full[:NCODE, :64] = vecs
    vext_full[:NCODE, 64] = 1.0
    vext_np = np.ascontiguousarray(
        vext_full.reshape(NCHUNK, 128, 65).transpose(1, 0, 2).reshape(128, NCHUNK * 65)
    ).astype(MMW_NP)

    embK = emb_spaces.reshape(K, N, E)
    w2 = np.einsum("kne,ked->knd", embK, lin_ws)  # [K,N,D]
    wxp_np = np.ascontiguousarray(w2.reshape(K * N, D).T)  # [64, 32]
    wemb_np = np.zeros((K * N, 68), dtype=np.float32)
    for k in range(K):
        wemb_np[k * N : (k + 1) * N, k * E : (k + 1) * E] = embK[k]
        wemb_np[k * N : (k + 1) * N, 64 + k] = 1.0
    wemb_np = wemb_np.astype(MMW_NP)

    x_flat = inputs.reshape(R, D)
    xT_cores = [
        np.ascontiguousarray(x_flat[c * RC : (c + 1) * RC].T) for c in range(NCORES)
    ]

    in_maps = [
        {
            "xT": xT_cores[c],
            "hgT": hg_cores[c],
            "hgpT": hgp_cores[c],
            "vT": vT_np,
            "vext": vext_np,
            "wxp": wxp_np,
            "wemb": wemb_np,
        }
        for c in range(NCORES)
    ]

    nc = _build_nc()
    res = run_bass_kernel_spmd(nc, in_maps, core_ids=list(range(NCORES)), trace=TRACE)
    LAST["exec_time_ns"] = res.exec_time_ns
    LAST["results"] = res

    quantized = np.empty((R, K * E), dtype=np.float32)
    quantized_coord = np.empty((R, D), dtype=np.float32)
    kl_c = 0.0
    kl_p = 0.0
    for c in range(NCORES):
        out = res.results[c]
        sl = slice(c * RC, (c + 1) * RC)

        qcT = out["qcT_o"].astype(np.float64)  # [65, RC]
        quantized_coord[sl] = (qcT[:64] / qcT[64:65]).T

        qpT = out["qpT_o"].astype(np.float64)  # [68, RC]
        se_p = qpT[64:68]  # [K, RC]
        qp = qpT[:64].reshape(K, E, RC) / se_p[:, None, :]
        quantized[sl] = qp.reshape(K * E, RC).T

        skl = out["sklT_o"].astype(np.float64)  # [65, RC]
        s1 = skl[64]
        s2 = np.einsum("dr,dr->r", xT_cores[c].astype(np.float64), skl[:64])
        # rows of kl_c: E_p[xc] - logsumexp - log_u
        kl_c += float(np.sum(s2 / s1 - (np.log(s1) + 2.0 * C3) - LOG_U_C))

        xp = out["xpT_o"].astype(np.float64).T.reshape(RC, K, N)
        m = xp.max(axis=-1, keepdims=True)
        ex = np.exp(xp - m)
        sp = ex.sum(axis=-1, keepdims=True)
        lse = m + np.log(sp)
        p = ex / sp
        kl_p += float(np.sum(p * (xp - lse - LOG_U_P)))

    loss = np.float32((kl_c + kl_p) / 5.0)
    return (
        quantized.reshape(B, S, K * E),
        quantized_coord.reshape(B, S, D),
        loss,
    )
